# revision 11
# baseline (speedup 1.0000x reference)
"""ExpertLoRA MoE kernel for 8x TRN2 NeuronCores (expert-parallel, routed).

Strategy
--------
The reference computes all 16 experts densely over all 1024 tokens and then
masks with the routing weights.  Only top-2 experts per token actually
contribute, so we:

  * host: fold LoRA into the main weights (W_eff = W + A@B*scaling — exact),
    de-interleave gate/up columns, compute per-expert routed token lists
    (weights of duplicate slots summed), gather + transpose tokens per expert
    into a runtime-sized capacity C (max expert load, rounded up to 8), and
    pack everything partition-major so every device DMA is a single large
    contiguous transfer (~1 MiB).
  * device (SPMD over 8 cores, 2 experts each): transposed-layout expert MLP
      guT = Wg^T @ xT ; upT = Wu^T @ xT           (PE, fp16 in / fp32 psum)
      gT  = act(guT, upT)                          (DVE + ACT)
      yT  = Wd^T @ gT                              (PE), cast fp16, DMA out
    All weights are SBUF-resident; every weight DMA is issued up-front in
    consumption order across the two HWDGE rings so HBM streams continuously
    at full rate.  A short burst of dummy matmuls at kernel start keeps the
    PE busy through the HAM warm-up window so real matmuls run at 2.4 GHz.
  * host: scatter-add  out[tok] += w * (y + bias)  per expert, plus an exact
    numpy fallback for the (practically impossible) case of an expert
    exceeding capacity.

Accuracy: fp16 matmul inputs with fp32 accumulation give ~3e-4 relative
absmax error end-to-end (measured).
"""
import numpy as np

E, H, F, R = 16, 1024, 1024, 16
D = 2 * F
TOPK = 2
SCALING = 16.0 / R
LIMIT = 7.0
ACT_ALPHA = 1.702
B_, S_ = 2, 512
T = B_ * S_
N_CORES = 8
EPC = E // N_CORES        # experts per core
KH = H // 128             # contraction tiles for H
KF = F // 128             # contraction tiles for F
MF = F // 128             # output tiles for F (gate or up half)
MH = H // 128             # output tiles for H
MP = MF // 2              # gate/up m-pairs per weight DMA block
HQ = MH // 4              # down h-quads per weight DMA block
C_MAX = 512               # hard cap (one PSUM bank); overflow -> host fallback
N_WARM = 18               # dummy matmuls: >=6.8us busy guarantees a fully-busy
                          # HAM window (free-running 3.4us) -> PE warm at 2.4GHz

DT_NAME = "float16"

_CACHE = {}


def _np_dt():
    import ml_dtypes
    return {"float16": np.float16, "float32r": np.float32,
            "float32": np.float32, "bfloat16": ml_dtypes.bfloat16}[DT_NAME]


def _build_nc(C):
    """Build the SPMD per-core Bass program (same NEFF for all 8 cores)."""
    import concourse.bass as bass
    import concourse.tile as tile
    import concourse.mybir as mybir
    from concourse import bacc

    DT = getattr(mybir.dt, DT_NAME)
    f32 = mybir.dt.float32
    AF = mybir.ActivationFunctionType
    OP = mybir.AluOpType

    nc = bacc.Bacc("TRN2", target_bir_lowering=False, debug=False,
                   enable_asserts=False, num_devices=N_CORES)

    # (p, e, mp, gu, mi, k, j): gate/up weights, one ~1MiB DMA per (e, mp)
    wgu_d = nc.dram_tensor("wgu", [128, EPC, MP, 2, 2, KH, 128], DT,
                           kind="ExternalInput").ap()
    # (p, e, hq, hi, k, j): down weights, one ~1MiB DMA per (e, hq)
    wd_d = nc.dram_tensor("wd", [128, EPC, HQ, 4, KF, 128], DT,
                          kind="ExternalInput").ap()
    xt_d = nc.dram_tensor("xt", [128, EPC, KH, C], DT, kind="ExternalInput").ap()
    # (p, e, which, m): which 0=gate bias, 1=up bias (+1 folded)
    bz_d = nc.dram_tensor("bz", [128, EPC, 2, 8], f32, kind="ExternalInput").ap()
    yt_d = nc.dram_tensor("yt", [128, EPC, MH, C], DT, kind="ExternalOutput").ap()
    dbg_d = nc.dram_tensor("dbg", [128, 8], f32, kind="ExternalOutput").ap()

    with tile.TileContext(nc) as tc:
        with tc.tile_pool(name="const", bufs=1) as const, \
             tc.tile_pool(name="wres", bufs=1) as wres, \
             tc.tile_pool(name="g", bufs=2) as gpool, \
             tc.tile_pool(name="act", bufs=4) as apool, \
             tc.tile_pool(name="y", bufs=2) as ypool, \
             tc.tile_pool(name="ps", bufs=7, space="PSUM") as pspool, \
             tc.tile_pool(name="wps", bufs=1, space="PSUM") as wpspool:

            # ---- all input DMAs up-front, in consumption order ----------
            # Everything goes on the nc.sync (SP) HWDGE queue: the ACT
            # engine's queue must stay free of DMA issues, or activations
            # queue up behind slow DMA_DIRECT2D instructions (FIFO engine
            # queues).  A single queue still uses all 16 SDMA engines.
            xt_sb = const.tile([128, EPC, KH, C], DT)
            bz_sb = const.tile([128, EPC, 2, 8], f32)
            nc.sync.dma_start(xt_sb[:, 0], xt_d[:, 0])

            wgu_t, wd_t = {}, {}
            blocks = []
            for e in range(EPC):
                for mp in range(MP):
                    blocks.append(("g", e, mp))
                for hq in range(HQ):
                    blocks.append(("d", e, hq))
            for i, (kind, e, j) in enumerate(blocks):
                if kind == "g":
                    t = wres.tile([128, 2, 2, KH, 128], DT, tag=f"wg{e}_{j}")
                    nc.sync.dma_start(t[:], wgu_d[:, e, j])
                    wgu_t[(e, j)] = t
                else:
                    # down blocks split in halves for a finer compute tail
                    t = wres.tile([128, 4, KF, 128], DT, tag=f"wd{e}_{j}")
                    nc.sync.dma_start(t[:, 0:2], wd_d[:, e, j, 0:2])
                    nc.sync.dma_start(t[:, 2:4], wd_d[:, e, j, 2:4])
                    wd_t[(e, j)] = t
                if i == 0:    # tiny bias DMA after b0 (don't break the ramp)
                    nc.sync.dma_start(bz_sb[:], bz_d)
                if i == MP:   # xt for expert 1 right after e0's gate/up blocks
                    nc.sync.dma_start(xt_sb[:, 1], xt_d[:, 1])

            # ---- ACT table preload: both tables load off the critical path
            # (reads uninitialized SBUF on purpose -- zero dependencies)
            wsink = const.tile([128, 8], f32)
            nc.scalar.activation(wsink[:], wsink[:], AF.Identity)
            nc.scalar.activation(wsink[:], wsink[:], AF.Gelu_apprx_sigmoid)

            # ---- PE warm-up: dummy matmuls while the first blocks stream --
            wz = const.tile([128, 512], DT)
            nc.vector.memset(wz[:], 0.0)
            warm_ps = wpspool.tile([128, 512], f32)
            for i in range(N_WARM):
                nc.tensor.matmul(warm_ps[:], wz[:, 0:128], wz[:],
                                 start=(i == 0), stop=(i == N_WARM - 1))
            nc.vector.tensor_copy(wsink[:], warm_ps[:, 0:8])
            nc.sync.dma_start(dbg_d, wsink[:])

            # ---- main expert loop ---------------------------------------
            def pe_filler(n):
                # dummy matmuls pace the PE to the DMA stream so it never
                # idles long enough for the HAM clock-gate to re-throttle
                for _ in range(n):
                    nc.tensor.matmul(warm_ps[:], wz[:, 0:128], wz[:],
                                     start=True, stop=True)

            for e in range(EPC):
                gT = gpool.tile([128, KF, C], DT, tag="gT")
                for mp in range(MP):
                    wgut = wgu_t[(e, mp)]
                    for mi in range(2):
                        m = 2 * mp + mi
                        psg = pspool.tile([128, C], f32, tag="ps")
                        psu = pspool.tile([128, C], f32, tag="ps")
                        for k in range(KH):
                            nc.tensor.matmul(psg[:], wgut[:, 0, mi, k],
                                             xt_sb[:, e, k],
                                             start=(k == 0), stop=(k == KH - 1))
                        for k in range(KH):
                            nc.tensor.matmul(psu[:], wgut[:, 1, mi, k],
                                             xt_sb[:, e, k],
                                             start=(k == 0), stop=(k == KH - 1))
                        # gate = min(psg + bg, 7)
                        gate = apool.tile([128, C], f32, tag="gate")
                        nc.vector.tensor_scalar(gate[:], psg[:],
                                                bz_sb[:, e, 0, m:m + 1], LIMIT,
                                                OP.add, OP.min)
                        # glu = gate * sigmoid(1.702 * gate)  (one ACT op)
                        glu = apool.tile([128, C], f32, tag="glu")
                        nc.scalar.activation(glu[:], gate[:],
                                             AF.Gelu_apprx_sigmoid)
                        # up1 = clip(psu + bu, -7, 7) + 1   (bu has +1 folded:
                        #   ACT adds bias, DVE clips to [-6, 8] in one op)
                        upb = apool.tile([128, C], f32, tag="upb")
                        nc.scalar.activation(upb[:], psu[:], AF.Identity,
                                             bias=bz_sb[:, e, 1, m:m + 1])
                        up = apool.tile([128, C], f32, tag="up")
                        nc.vector.tensor_scalar(up[:], upb[:],
                                                LIMIT + 1.0, -(LIMIT - 1.0),
                                                OP.min, OP.max)
                        # gT[:, m] = up1 * glu   (cast to DT)
                        nc.vector.tensor_mul(out=gT[:, m], in0=up[:], in1=glu[:])
                    pe_filler(2)
                yst = ypool.tile([128, MH, C], DT, tag="y")
                for hq in range(HQ):
                    wdt = wd_t[(e, hq)]
                    for hi in range(4):
                        h = 4 * hq + hi
                        psy = pspool.tile([128, C], f32, tag="ps")
                        for k in range(KF):
                            nc.tensor.matmul(psy[:], wdt[:, hi, k], gT[:, k],
                                             start=(k == 0), stop=(k == KF - 1))
                        nc.vector.tensor_copy(yst[:, h], psy[:])
                        if hi % 2 == 1:
                            h0 = h - 1
                            nc.sync.dma_start(yt_d[:, e, h0:h0 + 2],
                                              yst[:, h0:h0 + 2])
                        elif not (e == EPC - 1 and hq == HQ - 1):
                            pe_filler(1)
    nc.compile()
    return nc


def _get_nc(C):
    key = ("nc", C)
    if key not in _CACHE:
        _CACHE[key] = _build_nc(C)
    return _CACHE[key]


def _route(router_indices, routing_weights):
    """Per-expert unique token list + summed weights."""
    ri = np.asarray(router_indices)
    rw = np.asarray(routing_weights, dtype=np.float32)
    idxs, ws = [], []
    for e in range(E):
        m = ri == e
        any_m = m.any(axis=1)
        idx = np.nonzero(any_m)[0]
        w = (rw * m).sum(axis=1)[idx]
        idxs.append(idx.astype(np.int64))
        ws.append(w)
    return idxs, ws


def _fold_weights(gate_up_proj, gate_up_bias, down_proj, down_bias,
                  lora_gate_up_A, lora_gate_up_B, lora_down_A, lora_down_B):
    """LoRA-folded, gate/up-split, partition-major packed per-core tensors."""
    np_dt = _np_dt()
    gup = np.asarray(gate_up_proj, dtype=np.float32)
    gub = np.asarray(gate_up_bias, dtype=np.float32)
    dwn = np.asarray(down_proj, dtype=np.float32)
    Agu = np.asarray(lora_gate_up_A, dtype=np.float32)
    Bgu = np.asarray(lora_gate_up_B, dtype=np.float32)
    Ad = np.asarray(lora_down_A, dtype=np.float32)
    Bd = np.asarray(lora_down_B, dtype=np.float32)

    # W_eff = W + A @ B * s    (batched over experts)
    wgu = gup + np.einsum("ehr,erd->ehd", Agu, Bgu) * SCALING     # [E, H, D]
    wdn = dwn + np.einsum("efr,erh->efh", Ad, Bd) * SCALING       # [E, F, H]

    wg = wgu[:, :, 0::2]                                          # [E, H, F]
    wu = wgu[:, :, 1::2]
    bgs = gub[:, 0::2]                                            # [E, F]
    bus = gub[:, 1::2] + 1.0                                      # fold (+1)

    # gate/up combined: [E, p, mp, gu, mi, k, j]
    def prep(w):
        # [E, K*128, M*128] -> [E, k, p, m, j] -> [E, p, m, k, j]
        w = w.reshape(E, KH, 128, MF, 128).transpose(0, 2, 3, 1, 4)
        return w
    wgp = prep(wg).reshape(E, 128, MP, 2, KH, 128)
    wup = prep(wu).reshape(E, 128, MP, 2, KH, 128)
    wgu_all = np.stack([wgp, wup], axis=3)  # [E, 128, MP, gu, mi, k, j]
    wdp = wdn.reshape(E, KF, 128, MH, 128).transpose(0, 2, 3, 1, 4)
    wdp = wdp.reshape(E, 128, HQ, 4, KF, 128)

    # biases: [E, 128, 2, 8]
    bz = np.stack([
        bgs.reshape(E, MF, 128).transpose(0, 2, 1),
        bus.reshape(E, MF, 128).transpose(0, 2, 1),
    ], axis=2)

    wgu_cores, wd_cores, bz_cores = [], [], []
    for c in range(N_CORES):
        sl = slice(c * EPC, (c + 1) * EPC)
        wgu_cores.append(np.ascontiguousarray(
            wgu_all[sl].transpose(1, 0, 2, 3, 4, 5, 6), dtype=np_dt))
        wd_cores.append(np.ascontiguousarray(
            wdp[sl].transpose(1, 0, 2, 3, 4, 5), dtype=np_dt))
        bz_cores.append(np.ascontiguousarray(
            bz[sl].transpose(1, 0, 2, 3), dtype=np.float32))
    return {"wgu": wgu_cores, "wd": wd_cores, "bz": bz_cores}


def _expert_mlp_exact(x_e, Wg, Wu, bg, bu, Wd, bd):
    """fp32 numpy fallback (host) for capacity-overflow tokens."""
    gate = np.minimum(x_e @ Wg + bg, LIMIT)
    up = np.clip(x_e @ Wu + bu, -LIMIT, LIMIT)
    glu = gate / (1.0 + np.exp(-gate * ACT_ALPHA))
    g = (up + 1.0) * glu
    return g @ Wd + bd


def kernel(hidden_states, router_indices, routing_weights,
           gate_up_proj, gate_up_bias, down_proj, down_bias,
           lora_gate_up_A, lora_gate_up_B, lora_down_A, lora_down_B):
    from concourse import bass_utils

    np_dt = _np_dt()
    x = np.asarray(hidden_states, dtype=np.float32).reshape(T, H)
    idxs, ws = _route(router_indices, routing_weights)
    # runtime-specialized capacity: max expert load, rounded up to 8
    C = max(8, -(-max(len(i) for i in idxs) // 8) * 8)
    C = min(C, C_MAX)
    packed = _fold_weights(gate_up_proj, gate_up_bias, down_proj, down_bias,
                           lora_gate_up_A, lora_gate_up_B,
                           lora_down_A, lora_down_B)

    # gather + transpose tokens per expert: xt [128, EPC, KH, C]
    in_maps = []
    for c in range(N_CORES):
        xt = np.zeros((128, EPC, KH, C), dtype=np_dt)
        for j in range(EPC):
            e = c * EPC + j
            idx = idxs[e][:C]
            if len(idx):
                # x[idx] : [n, H] -> T -> [KH, 128, n] -> [128, KH, n]
                xg = x[idx].T.reshape(KH, 128, len(idx)).transpose(1, 0, 2)
                xt[:, j, :, :len(idx)] = xg.astype(np_dt)
        in_maps.append({
            "xt": xt,
            "wgu": packed["wgu"][c],
            "wd": packed["wd"][c],
            "bz": packed["bz"][c],
        })

    res = None
    try:
        nc = _get_nc(C)
        res = bass_utils.run_bass_kernel_spmd(
            nc, in_maps, core_ids=list(range(N_CORES)),
            **_CACHE.get("run_kwargs", {}))
    except Exception:
        try:
            nc = _get_nc(C)
            res = bass_utils.run_bass_kernel_spmd(
                nc, in_maps, core_ids=list(range(N_CORES)),
                **_CACHE.get("run_kwargs", {}))
        except Exception:
            res = None
    _CACHE["last_results"] = res
    if res is None:
        # device path failed: exact fp32 host fallback (slow but correct)
        out = np.zeros((T, H), dtype=np.float32)
        for e in range(E):
            idx = idxs[e]
            if not len(idx):
                continue
            gup = np.asarray(gate_up_proj[e], dtype=np.float32)
            Agu = np.asarray(lora_gate_up_A[e], dtype=np.float32)
            Bgu = np.asarray(lora_gate_up_B[e], dtype=np.float32)
            wgu = gup + Agu @ Bgu * SCALING
            dwn = np.asarray(down_proj[e], dtype=np.float32)
            Ad = np.asarray(lora_down_A[e], dtype=np.float32)
            Bd = np.asarray(lora_down_B[e], dtype=np.float32)
            wdn = dwn + Ad @ Bd * SCALING
            gub = np.asarray(gate_up_bias[e], dtype=np.float32)
            y = _expert_mlp_exact(x[idx], wgu[:, 0::2], wgu[:, 1::2],
                                  gub[0::2], gub[1::2], wdn,
                                  np.asarray(down_bias[e], dtype=np.float32))
            out[idx] += ws[e][:, None] * y
        return out.reshape(B_, S_, H)

    out = np.zeros((T, H), dtype=np.float32)
    for c in range(N_CORES):
        yt = res.results[c]["yt"]                   # [128, EPC, MH, C] fp16
        for j in range(EPC):
            e = c * EPC + j
            idx = idxs[e]
            n = min(len(idx), C)
            if n:
                # yt[p, j, h, t] -> y[t, h*128+p]  (+ down bias, host-side)
                y = yt[:, j, :, :n].transpose(2, 1, 0).reshape(n, H)
                y = y.astype(np.float32) + np.asarray(down_bias[e],
                                                      dtype=np.float32)
                out[idx[:n]] += ws[e][:n, None] * y
            if len(idx) > C:      # capacity overflow: exact host fallback
                gup = np.asarray(gate_up_proj[e], dtype=np.float32)
                Agu = np.asarray(lora_gate_up_A[e], dtype=np.float32)
                Bgu = np.asarray(lora_gate_up_B[e], dtype=np.float32)
                wgu = gup + Agu @ Bgu * SCALING
                dwn = np.asarray(down_proj[e], dtype=np.float32)
                Ad = np.asarray(lora_down_A[e], dtype=np.float32)
                Bd = np.asarray(lora_down_B[e], dtype=np.float32)
                wdn = dwn + Ad @ Bd * SCALING
                gub = np.asarray(gate_up_bias[e], dtype=np.float32)
                ovf = idx[C:]
                y2 = _expert_mlp_exact(x[ovf], wgu[:, 0::2], wgu[:, 1::2],
                                       gub[0::2], gub[1::2],
                                       wdn, np.asarray(down_bias[e],
                                                       dtype=np.float32))
                out[ovf] += ws[e][C:, None] * y2
    return out.reshape(B_, S_, H)


# revision 16
# speedup vs baseline: 1.0159x; 1.0159x over previous
"""ExpertLoRA MoE kernel for 8x TRN2 NeuronCores (expert-parallel, routed).

Strategy
--------
The reference computes all 16 experts densely over all 1024 tokens and then
masks with the routing weights.  Only top-2 experts per token actually
contribute, so we:

  * host: fold LoRA into the main weights (W_eff = W + A@B*scaling — exact),
    de-interleave gate/up columns, compute per-expert routed token lists
    (weights of duplicate slots summed), gather + transpose tokens per expert
    into a runtime-sized capacity C (max expert load, rounded up to 8), and
    pack everything partition-major so every device DMA is a single large
    contiguous transfer (~1 MiB).
  * device (SPMD over 8 cores, 2 experts each): transposed-layout expert MLP
      guT = Wg^T @ xT ; upT = Wu^T @ xT           (PE, fp16 in / fp32 psum)
      gT  = act(guT, upT)                          (DVE + ACT)
      yT  = Wd^T @ gT                              (PE), cast fp16, DMA out
    All weights are SBUF-resident; every weight DMA is issued up-front in
    consumption order across the two HWDGE rings so HBM streams continuously
    at full rate.  A short burst of dummy matmuls at kernel start keeps the
    PE busy through the HAM warm-up window so real matmuls run at 2.4 GHz.
  * host: scatter-add  out[tok] += w * (y + bias)  per expert, plus an exact
    numpy fallback for the (practically impossible) case of an expert
    exceeding capacity.

Accuracy: fp16 matmul inputs with fp32 accumulation give ~3e-4 relative
absmax error end-to-end (measured).
"""
import numpy as np

E, H, F, R = 16, 1024, 1024, 16
D = 2 * F
TOPK = 2
SCALING = 16.0 / R
LIMIT = 7.0
ACT_ALPHA = 1.702
B_, S_ = 2, 512
T = B_ * S_
N_CORES = 8
EPC = E // N_CORES        # experts per core
KH = H // 128             # contraction tiles for H
KF = F // 128             # contraction tiles for F
MF = F // 128             # output tiles for F (gate or up half)
MH = H // 128             # output tiles for H
MP = MF // 2              # gate/up m-pairs per weight DMA block
HQ = MH // 4              # down h-quads per weight DMA block
C_MAX = 512               # hard cap (one PSUM bank); overflow -> host fallback
N_WARM = 8                # dummy matmuls bridging the PE to the first block

# ---- pacing model for filler placement (ns) -------------------------------
SIM_RATE = 400.0          # DMA bytes/ns while streaming (~400 GB/s observed)
SIM_DMA0 = 1400           # first-byte latency after user code starts
SIM_MM = 75               # effective per-matmul pace (N~144, warm)
SIM_FILL = 213            # one N=512 filler matmul
SIM_WARM_MM = 427         # cold N=512 warmup matmul

DT_NAME = "float16"

_CACHE = {}


def _np_dt():
    import ml_dtypes
    return {"float16": np.float16, "float32r": np.float32,
            "float32": np.float32, "bfloat16": ml_dtypes.bfloat16}[DT_NAME]


def _build_nc(C):
    """Build the SPMD per-core Bass program (same NEFF for all 8 cores)."""
    import concourse.bass as bass
    import concourse.tile as tile
    import concourse.mybir as mybir
    from concourse import bacc

    DT = getattr(mybir.dt, DT_NAME)
    f32 = mybir.dt.float32
    AF = mybir.ActivationFunctionType
    OP = mybir.AluOpType

    nc = bacc.Bacc("TRN2", target_bir_lowering=False, debug=False,
                   enable_asserts=False, num_devices=N_CORES)

    # (p, e, mp, gu, mi, k, j): gate/up weights, one ~1MiB DMA per (e, mp)
    wgu_d = nc.dram_tensor("wgu", [128, EPC, MP, 2, 2, KH, 128], DT,
                           kind="ExternalInput").ap()
    # (p, e, hq, hi, k, j): down weights, one ~1MiB DMA per (e, hq)
    wd_d = nc.dram_tensor("wd", [128, EPC, HQ, 4, KF, 128], DT,
                          kind="ExternalInput").ap()
    xt_d = nc.dram_tensor("xt", [128, EPC, KH, C], DT, kind="ExternalInput").ap()
    # (p, e, which, m): which 0=gate bias, 1=up bias (+1 folded)
    bz_d = nc.dram_tensor("bz", [128, EPC, 2, 8], f32, kind="ExternalInput").ap()
    yt_d = nc.dram_tensor("yt", [128, EPC, MH, C], DT, kind="ExternalOutput").ap()
    dbg_d = nc.dram_tensor("dbg", [128, 8], f32, kind="ExternalOutput").ap()

    with tile.TileContext(nc) as tc:
        with tc.tile_pool(name="const", bufs=1) as const, \
             tc.tile_pool(name="wres", bufs=1) as wres, \
             tc.tile_pool(name="g", bufs=2) as gpool, \
             tc.tile_pool(name="act", bufs=4) as apool, \
             tc.tile_pool(name="y", bufs=2) as ypool, \
             tc.tile_pool(name="ps", bufs=7, space="PSUM") as pspool, \
             tc.tile_pool(name="wps", bufs=1, space="PSUM") as wpspool:

            # ---- all input DMAs up-front, in consumption order ----------
            # Everything goes on the nc.sync (SP) HWDGE queue: the ACT
            # engine's queue must stay free of DMA issues, or activations
            # queue up behind slow DMA_DIRECT2D instructions (FIFO engine
            # queues).  A single queue still uses all 16 SDMA engines.
            xt_sb = const.tile([128, EPC, KH, C], DT)
            bz_sb = const.tile([128, EPC, 2, 8], f32)
            nc.sync.dma_start(xt_sb[:, 0], xt_d[:, 0])
            cum = [128 * KH * C * 2]

            # consumption units in stream order; each entry:
            #   ("gu", e, mp, mi) -> 16 matmuls   (0.525 MB DMA)
            #   ("d",  e, hq, half) -> 16 matmuls (0.525 MB DMA)
            arrive = {}

            def track(key, nbytes):
                cum[0] += nbytes
                arrive[key] = SIM_DMA0 + cum[0] / SIM_RATE

            wgu_t, wd_t = {}, {}
            for e in range(EPC):
                for mp in range(MP):
                    t = wres.tile([128, 2, 2, KH, 128], DT, tag=f"wg{e}_{mp}")
                    for mi in range(2):
                        nc.sync.dma_start(t[:, :, mi], wgu_d[:, e, mp, :, mi])
                        track(("gu", e, mp, mi), 128 * 2 * KH * 128 * 2)
                    wgu_t[(e, mp)] = t
                    if e == 0 and mp == 0:   # tiny bias DMA after first block
                        nc.sync.dma_start(bz_sb[:], bz_d)
                        cum[0] += 128 * EPC * 2 * 8 * 4
                for hq in range(HQ):
                    t = wres.tile([128, 4, KF, 128], DT, tag=f"wd{e}_{hq}")
                    for half in range(2):
                        nc.sync.dma_start(t[:, 2 * half:2 * half + 2],
                                          wd_d[:, e, hq, 2 * half:2 * half + 2])
                        track(("d", e, hq, half), 128 * 2 * KF * 128 * 2)
                    wd_t[(e, hq)] = t
                if e == 0:    # xt for expert 1 after all of e0's weights
                    nc.sync.dma_start(xt_sb[:, 1], xt_d[:, 1])
                    cum[0] += 128 * KH * C * 2

            # ---- filler plan: simulate PE vs DMA arrivals ---------------
            units = []
            for e in range(EPC):
                for mp in range(MP):
                    for mi in range(2):
                        units.append(("gu", e, mp, mi))
                for hq in range(HQ):
                    for half in range(2):
                        units.append(("d", e, hq, half))
            fillers = {}
            t_pe = 900 + N_WARM * SIM_WARM_MM
            mm_t = SIM_MM * C / 144.0
            for ui, key in enumerate(units):
                gap = arrive[key] - t_pe
                n_fill = 0
                if gap > 250 and ui < len(units) - 3:
                    n_fill = min(6, int(0.85 * gap / SIM_FILL))
                fillers[key] = n_fill
                t_pe += n_fill * SIM_FILL
                t_pe = max(t_pe, arrive[key]) + 16 * mm_t

            # ---- ACT table preload: both tables load off the critical path
            # (reads uninitialized SBUF on purpose -- zero dependencies)
            wsink = const.tile([128, 8], f32)
            nc.scalar.activation(wsink[:], wsink[:], AF.Identity)
            nc.scalar.activation(wsink[:], wsink[:], AF.Gelu_apprx_sigmoid)

            # ---- PE warm-up: dummy matmuls while the first blocks stream --
            wz = const.tile([128, 512], DT)
            nc.vector.memset(wz[:], 0.0)
            warm_ps = wpspool.tile([128, 512], f32)
            for i in range(N_WARM):
                nc.tensor.matmul(warm_ps[:], wz[:, 0:128], wz[:],
                                 start=(i == 0), stop=(i == N_WARM - 1))
            nc.vector.tensor_copy(wsink[:], warm_ps[:, 0:8])
            nc.sync.dma_start(dbg_d, wsink[:])

            # ---- main expert loop ---------------------------------------
            def pe_filler(n):
                # dummy matmuls pace the PE to the DMA stream so it never
                # idles long enough for the HAM clock-gate to re-throttle
                for _ in range(n):
                    nc.tensor.matmul(warm_ps[:], wz[:, 0:128], wz[:],
                                     start=True, stop=True)

            for e in range(EPC):
                gT = gpool.tile([128, KF, C], DT, tag="gT")
                for mp in range(MP):
                    wgut = wgu_t[(e, mp)]
                    for mi in range(2):
                        m = 2 * mp + mi
                        pe_filler(fillers[("gu", e, mp, mi)])
                        psg = pspool.tile([128, C], f32, tag="ps")
                        psu = pspool.tile([128, C], f32, tag="ps")
                        for k in range(KH):
                            nc.tensor.matmul(psg[:], wgut[:, 0, mi, k],
                                             xt_sb[:, e, k],
                                             start=(k == 0), stop=(k == KH - 1))
                        for k in range(KH):
                            nc.tensor.matmul(psu[:], wgut[:, 1, mi, k],
                                             xt_sb[:, e, k],
                                             start=(k == 0), stop=(k == KH - 1))
                        # gate = min(psg + bg, 7)
                        gate = apool.tile([128, C], f32, tag="gate")
                        nc.vector.tensor_scalar(gate[:], psg[:],
                                                bz_sb[:, e, 0, m:m + 1], LIMIT,
                                                OP.add, OP.min)
                        # glu = gate * sigmoid(1.702 * gate)  (one ACT op)
                        glu = apool.tile([128, C], f32, tag="glu")
                        nc.scalar.activation(glu[:], gate[:],
                                             AF.Gelu_apprx_sigmoid)
                        # up1 = clip(psu + bu, -7, 7) + 1   (bu has +1 folded:
                        #   ACT adds bias, DVE clips to [-6, 8] in one op)
                        upb = apool.tile([128, C], f32, tag="upb")
                        nc.scalar.activation(upb[:], psu[:], AF.Identity,
                                             bias=bz_sb[:, e, 1, m:m + 1])
                        up = apool.tile([128, C], f32, tag="up")
                        nc.vector.tensor_scalar(up[:], upb[:],
                                                LIMIT + 1.0, -(LIMIT - 1.0),
                                                OP.min, OP.max)
                        # gT[:, m] = up1 * glu   (cast to DT)
                        nc.vector.tensor_mul(out=gT[:, m], in0=up[:], in1=glu[:])
                yst = ypool.tile([128, MH, C], DT, tag="y")
                for hq in range(HQ):
                    wdt = wd_t[(e, hq)]
                    for hi in range(4):
                        h = 4 * hq + hi
                        if hi % 2 == 0:
                            pe_filler(fillers[("d", e, hq, hi // 2)])
                        psy = pspool.tile([128, C], f32, tag="ps")
                        for k in range(KF):
                            nc.tensor.matmul(psy[:], wdt[:, hi, k], gT[:, k],
                                             start=(k == 0), stop=(k == KF - 1))
                        nc.vector.tensor_copy(yst[:, h], psy[:])
                        if hi % 2 == 1:
                            h0 = h - 1
                            nc.sync.dma_start(yt_d[:, e, h0:h0 + 2],
                                              yst[:, h0:h0 + 2])
    nc.compile()
    return nc


def _get_nc(C):
    key = ("nc", C)
    if key not in _CACHE:
        _CACHE[key] = _build_nc(C)
    return _CACHE[key]


def _route(router_indices, routing_weights):
    """Per-expert unique token list + summed weights."""
    ri = np.asarray(router_indices)
    rw = np.asarray(routing_weights, dtype=np.float32)
    idxs, ws = [], []
    for e in range(E):
        m = ri == e
        any_m = m.any(axis=1)
        idx = np.nonzero(any_m)[0]
        w = (rw * m).sum(axis=1)[idx]
        idxs.append(idx.astype(np.int64))
        ws.append(w)
    return idxs, ws


def _fold_weights(gate_up_proj, gate_up_bias, down_proj, down_bias,
                  lora_gate_up_A, lora_gate_up_B, lora_down_A, lora_down_B):
    """LoRA-folded, gate/up-split, partition-major packed per-core tensors."""
    np_dt = _np_dt()
    gup = np.asarray(gate_up_proj, dtype=np.float32)
    gub = np.asarray(gate_up_bias, dtype=np.float32)
    dwn = np.asarray(down_proj, dtype=np.float32)
    Agu = np.asarray(lora_gate_up_A, dtype=np.float32)
    Bgu = np.asarray(lora_gate_up_B, dtype=np.float32)
    Ad = np.asarray(lora_down_A, dtype=np.float32)
    Bd = np.asarray(lora_down_B, dtype=np.float32)

    # W_eff = W + A @ B * s    (batched over experts)
    wgu = gup + np.einsum("ehr,erd->ehd", Agu, Bgu) * SCALING     # [E, H, D]
    wdn = dwn + np.einsum("efr,erh->efh", Ad, Bd) * SCALING       # [E, F, H]

    wg = wgu[:, :, 0::2]                                          # [E, H, F]
    wu = wgu[:, :, 1::2]
    bgs = gub[:, 0::2]                                            # [E, F]
    bus = gub[:, 1::2] + 1.0                                      # fold (+1)

    # gate/up combined: [E, p, mp, gu, mi, k, j]
    def prep(w):
        # [E, K*128, M*128] -> [E, k, p, m, j] -> [E, p, m, k, j]
        w = w.reshape(E, KH, 128, MF, 128).transpose(0, 2, 3, 1, 4)
        return w
    wgp = prep(wg).reshape(E, 128, MP, 2, KH, 128)
    wup = prep(wu).reshape(E, 128, MP, 2, KH, 128)
    wgu_all = np.stack([wgp, wup], axis=3)  # [E, 128, MP, gu, mi, k, j]
    wdp = wdn.reshape(E, KF, 128, MH, 128).transpose(0, 2, 3, 1, 4)
    wdp = wdp.reshape(E, 128, HQ, 4, KF, 128)

    # biases: [E, 128, 2, 8]
    bz = np.stack([
        bgs.reshape(E, MF, 128).transpose(0, 2, 1),
        bus.reshape(E, MF, 128).transpose(0, 2, 1),
    ], axis=2)

    wgu_cores, wd_cores, bz_cores = [], [], []
    for c in range(N_CORES):
        sl = slice(c * EPC, (c + 1) * EPC)
        wgu_cores.append(np.ascontiguousarray(
            wgu_all[sl].transpose(1, 0, 2, 3, 4, 5, 6), dtype=np_dt))
        wd_cores.append(np.ascontiguousarray(
            wdp[sl].transpose(1, 0, 2, 3, 4, 5), dtype=np_dt))
        bz_cores.append(np.ascontiguousarray(
            bz[sl].transpose(1, 0, 2, 3), dtype=np.float32))
    return {"wgu": wgu_cores, "wd": wd_cores, "bz": bz_cores}


def _expert_mlp_exact(x_e, Wg, Wu, bg, bu, Wd, bd):
    """fp32 numpy fallback (host) for capacity-overflow tokens."""
    gate = np.minimum(x_e @ Wg + bg, LIMIT)
    up = np.clip(x_e @ Wu + bu, -LIMIT, LIMIT)
    glu = gate / (1.0 + np.exp(-gate * ACT_ALPHA))
    g = (up + 1.0) * glu
    return g @ Wd + bd


def kernel(hidden_states, router_indices, routing_weights,
           gate_up_proj, gate_up_bias, down_proj, down_bias,
           lora_gate_up_A, lora_gate_up_B, lora_down_A, lora_down_B):
    from concourse import bass_utils

    np_dt = _np_dt()
    x = np.asarray(hidden_states, dtype=np.float32).reshape(T, H)
    idxs, ws = _route(router_indices, routing_weights)
    # runtime-specialized capacity: max expert load, rounded up to 8
    C = max(8, -(-max(len(i) for i in idxs) // 8) * 8)
    C = min(C, C_MAX)
    packed = _fold_weights(gate_up_proj, gate_up_bias, down_proj, down_bias,
                           lora_gate_up_A, lora_gate_up_B,
                           lora_down_A, lora_down_B)

    # gather + transpose tokens per expert: xt [128, EPC, KH, C]
    in_maps = []
    for c in range(N_CORES):
        xt = np.zeros((128, EPC, KH, C), dtype=np_dt)
        for j in range(EPC):
            e = c * EPC + j
            idx = idxs[e][:C]
            if len(idx):
                # x[idx] : [n, H] -> T -> [KH, 128, n] -> [128, KH, n]
                xg = x[idx].T.reshape(KH, 128, len(idx)).transpose(1, 0, 2)
                xt[:, j, :, :len(idx)] = xg.astype(np_dt)
        in_maps.append({
            "xt": xt,
            "wgu": packed["wgu"][c],
            "wd": packed["wd"][c],
            "bz": packed["bz"][c],
        })

    res = None
    try:
        nc = _get_nc(C)
        res = bass_utils.run_bass_kernel_spmd(
            nc, in_maps, core_ids=list(range(N_CORES)),
            **_CACHE.get("run_kwargs", {}))
    except Exception:
        try:
            nc = _get_nc(C)
            res = bass_utils.run_bass_kernel_spmd(
                nc, in_maps, core_ids=list(range(N_CORES)),
                **_CACHE.get("run_kwargs", {}))
        except Exception:
            res = None
    _CACHE["last_results"] = res
    if res is None:
        # device path failed: exact fp32 host fallback (slow but correct)
        out = np.zeros((T, H), dtype=np.float32)
        for e in range(E):
            idx = idxs[e]
            if not len(idx):
                continue
            gup = np.asarray(gate_up_proj[e], dtype=np.float32)
            Agu = np.asarray(lora_gate_up_A[e], dtype=np.float32)
            Bgu = np.asarray(lora_gate_up_B[e], dtype=np.float32)
            wgu = gup + Agu @ Bgu * SCALING
            dwn = np.asarray(down_proj[e], dtype=np.float32)
            Ad = np.asarray(lora_down_A[e], dtype=np.float32)
            Bd = np.asarray(lora_down_B[e], dtype=np.float32)
            wdn = dwn + Ad @ Bd * SCALING
            gub = np.asarray(gate_up_bias[e], dtype=np.float32)
            y = _expert_mlp_exact(x[idx], wgu[:, 0::2], wgu[:, 1::2],
                                  gub[0::2], gub[1::2], wdn,
                                  np.asarray(down_bias[e], dtype=np.float32))
            out[idx] += ws[e][:, None] * y
        return out.reshape(B_, S_, H)

    out = np.zeros((T, H), dtype=np.float32)
    for c in range(N_CORES):
        yt = res.results[c]["yt"]                   # [128, EPC, MH, C] fp16
        for j in range(EPC):
            e = c * EPC + j
            idx = idxs[e]
            n = min(len(idx), C)
            if n:
                # yt[p, j, h, t] -> y[t, h*128+p]  (+ down bias, host-side)
                y = yt[:, j, :, :n].transpose(2, 1, 0).reshape(n, H)
                y = y.astype(np.float32) + np.asarray(down_bias[e],
                                                      dtype=np.float32)
                out[idx[:n]] += ws[e][:n, None] * y
            if len(idx) > C:      # capacity overflow: exact host fallback
                gup = np.asarray(gate_up_proj[e], dtype=np.float32)
                Agu = np.asarray(lora_gate_up_A[e], dtype=np.float32)
                Bgu = np.asarray(lora_gate_up_B[e], dtype=np.float32)
                wgu = gup + Agu @ Bgu * SCALING
                dwn = np.asarray(down_proj[e], dtype=np.float32)
                Ad = np.asarray(lora_down_A[e], dtype=np.float32)
                Bd = np.asarray(lora_down_B[e], dtype=np.float32)
                wdn = dwn + Ad @ Bd * SCALING
                gub = np.asarray(gate_up_bias[e], dtype=np.float32)
                ovf = idx[C:]
                y2 = _expert_mlp_exact(x[ovf], wgu[:, 0::2], wgu[:, 1::2],
                                       gub[0::2], gub[1::2],
                                       wdn, np.asarray(down_bias[e],
                                                       dtype=np.float32))
                out[ovf] += ws[e][C:, None] * y2
    return out.reshape(B_, S_, H)
